# revision 49
# baseline (speedup 1.0000x reference)
"""NeuralSort P_hat @ scores kernel for Trainium2 (8 NeuronCores, data-parallel over batch).

Math (per batch row s[0:N], N=1024):
  r_j   = sum_k |s_j - s_k|
  a_i   = N + 1 - 2*(i+1) = 1023 - 2i
  t_ij  = a_i * s_j - r_j
  out_i = sum_j softmax_j(t_i)_j * s_j

Per-core plan (8 batches/core), fused on-chip. All matmuls fp32r (1 cyc/row
at F>=256). Operands needing >12 mantissa bits are hi/lo split; K-dim pairs
reassemble full precision in the fp32 PSUM accumulation.

v3 structure:
  - s broadcast ([P, N] tile with s_k on every partition) via SWDGE-queue
    DMA issued at kernel start: latency fully hidden, no engine cost.
  - rowsum r_j: ACT abs+bias+accum (1 op/tile) and DVE sub + abs-add-reduce
    (2 ops/tile), split to balance the engines.
  - column->row conversion (lhsT6 data rows, nm anchor rows) via PE
    transpose matmuls + small PSUM->SBUF copies: no DRAM scatter roundtrips.
  - phase 3 per j-tile: K=6 fp32r matmul pair builds
    t[j,i] = a_i*s_j - r_j - m_i in PSUM, ACT exp, K=128 fp32r accumulating
    matmul pair -> (numer_hi, numer_lo, denom).
  - P2 anchor bound: f(a)=max_j(a*s_j - r_j) convex in a; 128 anchors at
    i=8g upper-bound the row max (slack <= 64, harmless to the softmax).
  - program order pipelines batches 4-7's phase A under batches 0-3's
    phase 3; rotating 4-slot lhsT6/rhs6 tiles (slot writes for batch b+4
    emitted only after phase3(b)).
"""

import numpy as np
from contextlib import ExitStack

try:
    import concourse.bass as bass
except ImportError:
    import sys
    sys.path.insert(0, "/opt/trn_rl_repo")
    import concourse.bass as bass
import concourse.tile as tile
from concourse import bacc, mybir, masks
from concourse.bass_utils import run_bass_kernel_spmd

B, N = 64, 1024
NCORES = 8
BPC = B // NCORES      # batches per core
P = 128                # partitions
NT = N // P            # 8 tiles of 128
F32 = mybir.dt.float32
F32R = mybir.dt.float32r
AX = mybir.AxisListType
OP = mybir.AluOpType
ACT = mybir.ActivationFunctionType
H = N // 2             # matmul moving-dim half

ACT_G1 = (0, 1, 2, 3)  # rowsum tiles on ACT while it is mostly idle
ACT_G2 = (0, 1, 2, 3, 4)  # rowsum tiles on ACT while it also runs exp


def build_kernel():
    nc = bacc.Bacc("TRN2", target_bir_lowering=False, debug=False)

    scores = nc.dram_tensor("scores", [BPC, N], F32, kind="ExternalInput").ap()
    avals = nc.dram_tensor("avals", [1, N], F32, kind="ExternalInput").ap()
    onesv = nc.dram_tensor("onesv", [1, N], F32, kind="ExternalInput").ap()
    out_ext = nc.dram_tensor("out", [BPC, N], F32, kind="ExternalOutput").ap()

    with tile.TileContext(nc) as tc, ExitStack() as ctx:
        const = ctx.enter_context(tc.tile_pool(name="const", bufs=1))
        small = ctx.enter_context(tc.tile_pool(name="small", bufs=4))
        sp = ctx.enter_context(tc.tile_pool(name="sp", bufs=3))
        epool = ctx.enter_context(tc.tile_pool(name="epool", bufs=4))
        psum_tb = ctx.enter_context(tc.tile_pool(name="psum_tb", bufs=2, space="PSUM"))
        psum_acc = ctx.enter_context(tc.tile_pool(name="psum_acc", bufs=1, space="PSUM"))
        psum_an = ctx.enter_context(tc.tile_pool(name="psum_an", bufs=2, space="PSUM"))

        # ---- input load first: rowsum(0) is on the critical path ----
        # contiguous row load + PE transposes (a direct [p,b,t] gather DMA
        # is a 4-byte-element scatter that costs ~16us)
        s_rows = const.tile([BPC, N], F32)
        nc.sync.dma_start(out=s_rows, in_=scores)
        s_cols_all = const.tile([P, BPC, NT], F32)   # s_cols[p,b,t]=s[b,t*P+p]

        # ---- constants ----
        # a_anch_r: K=4 lhsT for P2 anchors, rows (a_{8g}, a_{8g}, 1, 1)
        a4f = const.tile([4, P], F32)
        aview = bass.AP(tensor=avals.tensor, offset=avals.offset,
                        ap=[[0, 1], [8, P]])
        nc.sync.dma_start(out=a4f[0:1, :], in_=aview)
        nc.sync.dma_start(out=a4f[1:2, :], in_=aview)
        nc.sync.dma_start(out=a4f[2:3, :], in_=onesv[0:1, 0:P])
        nc.sync.dma_start(out=a4f[3:4, :], in_=onesv[0:1, 0:P])
        a_anch_r = const.tile([4, P], F32R)
        nc.vector.tensor_copy(a_anch_r, a4f)
        # c4_r rows (a, a, 1, 1): the constant head of every rhs6
        c4f = const.tile([4, N], F32)
        nc.sync.dma_start(out=c4f[0:1, :], in_=avals)
        nc.sync.dma_start(out=c4f[1:2, :], in_=avals)
        nc.sync.dma_start(out=c4f[2:3, :], in_=onesv)
        nc.sync.dma_start(out=c4f[3:4, :], in_=onesv)
        c4_r = const.tile([4, N], F32R)
        nc.vector.tensor_copy(c4_r, c4f)
        ones2f = const.tile([2, N], F32)
        nc.vector.memset(ones2f, 1.0)
        ones2 = const.tile([2, N], F32R)
        nc.vector.tensor_copy(ones2, ones2f)
        # identity for PE transposes (f32 bits == f32r bits for 0/1)
        idf = const.tile([P, P], F32)
        masks.make_identity(nc, idf[:, :])
        idr = const.tile([P, P], F32R)
        nc.vector.tensor_copy(idr, idf)
        # rotating per-batch lhsT6/rhs6 tiles (PE needs base partition 0):
        # lhsT6 rows 0..3 = (s_hi, s_lo, nr_hi, nr_lo), rows 4,5 = 1
        # rhs6 rows 0..3 = (a, a, 1, 1), rows 4,5 = (nm_hi, nm_lo)
        lhsT6_tiles = [const.tile([6, N], F32R, tag=f"lhsT6_{q}",
                                  name=f"lhsT6_{q}") for q in range(4)]
        rhs6_tiles = [const.tile([6, N], F32R, tag=f"rhs6_{q}",
                                 name=f"rhs6_{q}") for q in range(4)]
        for q in range(4):
            nc.sync.dma_start(out=rhs6_tiles[q][0:4, :], in_=c4_r)
            nc.sync.dma_start(out=lhsT6_tiles[q][4:6, :], in_=ones2)

        # se_all[:, b, t, c]: acc-matmul lhsT columns (s_hi, s_lo, 1)
        se_all = const.tile([P, BPC, NT, 3], F32R)
        ones_cols64f = const.tile([P, BPC, NT], F32)
        nc.vector.memset(ones_cols64f, 1.0)
        nc.vector.tensor_copy(se_all[:, :, :, 2], ones_cols64f)

        # s_cols via PE transpose: [8, 128]@t -> [128, 8] -> SBUF
        for t in range(NT):
            trc = psum_an.tile([P, H], F32, tag="an")
            nc.tensor.matmul(trc[0:P, 0:BPC], s_rows[:, t * P:(t + 1) * P],
                             idf[0:BPC, 0:BPC], is_transpose=True,
                             start=True, stop=True)
            nc.vector.tensor_copy(s_cols_all[:, :, t], trc[0:P, 0:BPC])

        # ---- persistent per-batch tiles ----
        s_bcast_all = const.tile([P, BPC, N], F32)
        hilo_all = const.tile([P, BPC, 4, NT], F32R)  # (s_hi|s_lo|nr_hi|nr_lo)
        r_cols_all = const.tile([P, BPC, NT], F32)
        ns_cols_all = const.tile([P, BPC, NT], F32)
        nmA_part_all = const.tile([P, BPC, 2], F32)
        nmA_cols_all = const.tile([P, BPC, 1], F32)
        numer_hi_all = const.tile([BPC, N], F32)
        numer_lo_all = const.tile([BPC, N], F32)
        denom_all = const.tile([BPC, N], F32)

        # ---- phase helpers ----
        def bcast_dma(b, eng):
            # s_bcast[p, k] = s_k for all p: stride-0 DRAM broadcast -- slow
            # (~10us) but issued early so its latency is fully hidden and it
            # costs no compute-engine time
            src = scores[b:b + 1, :]
            eng.dma_start(out=s_bcast_all[:, b, :], in_=bass.AP(
                tensor=src.tensor, offset=src.offset, ap=[[0, P], [1, N]]))

        def splits_all():
            # one strided op per derived tensor, covering all 8 batches
            nc.vector.tensor_copy(hilo_all[:, :, 0, :], s_cols_all)
            nc.vector.tensor_sub(hilo_all[:, :, 1, :], s_cols_all,
                                 hilo_all[:, :, 0, :].bitcast(F32))
            nc.vector.tensor_copy(se_all[:, :, :, 0], hilo_all[:, :, 0, :])
            nc.vector.tensor_copy(se_all[:, :, :, 1], hilo_all[:, :, 1, :])
            nc.vector.tensor_scalar_mul(ns_cols_all, s_cols_all, -1.0)

        def act_tile(b, jt):
            scratch = sp.tile([P, N], F32, tag="rs")
            nc.scalar.activation(
                out=scratch, in_=s_bcast_all[:, b, :], func=ACT.Abs,
                bias=ns_cols_all[:, b, jt:jt + 1], scale=1.0,
                accum_out=r_cols_all[:, b, jt:jt + 1])

        def rowsum(b, act_tiles=(), defer=False):
            # r_j = sum_k |s_k - s_j|; ACT tiles optionally deferred so the
            # caller can weave them into phase3's exp stream
            deferred = []
            for jt in range(NT):
                if jt in act_tiles:
                    if defer:
                        deferred.append((b, jt))
                    else:
                        act_tile(b, jt)
                else:
                    sb = s_bcast_all[:, b, :]
                    scratch = sp.tile([P, N], F32, tag="rs")
                    nc.vector.tensor_scalar(
                        out=scratch, in0=sb,
                        scalar1=s_cols_all[:, b, jt:jt + 1], scalar2=None,
                        op0=OP.subtract, op1=OP.bypass)
                    nc.vector.tensor_reduce(
                        out=r_cols_all[:, b, jt:jt + 1], in_=scratch,
                        axis=AX.X, op=OP.add, apply_absolute_value=True)
            return deferred

        def nr_split(b):
            nrc = small.tile([P, NT], F32, tag="nrc")
            nc.vector.tensor_scalar_mul(nrc, r_cols_all[:, b, :], -1.0)
            nc.vector.tensor_copy(hilo_all[:, b, 2, :], nrc)
            nc.vector.tensor_sub(hilo_all[:, b, 3, :], nrc,
                                 hilo_all[:, b, 2, :].bitcast(F32))

        def make_rows(b, q0=0, qn=4):
            # lhsT6 rows q0..q0+qn per jt via PE transpose of hilo columns:
            # [128, qn] block @ jt -> PSUM [qn, 128] -> SBUF row slices
            dst = lhsT6_tiles[b % 4]
            for jt in range(NT):
                tr = psum_an.tile([P, H], F32, tag="an")
                trv = tr[0:qn, 0:P].bitcast(F32R)
                nc.tensor.matmul(trv, hilo_all[:, b, q0:q0 + qn, jt],
                                 idr, is_transpose=True,
                                 start=True, stop=True)
                nc.vector.tensor_copy(dst[q0:q0 + qn, jt * P:(jt + 1) * P],
                                      trv)

        def anchors(b):
            # P2: f(a)=max_j(a*s_j - r_j) is convex in a, so on each anchor
            # interval max(f(a_g), f(a_{g+1})) >= f(a); 128 anchors at i=8g.
            rhs4 = lhsT6_tiles[b % 4][0:4, :]
            for h in range(2):
                tAn = psum_an.tile([P, H], F32, tag="an")
                nc.tensor.matmul(tAn, a_anch_r, rhs4[:, h * H:(h + 1) * H],
                                 start=True, stop=True)
                nc.vector.tensor_reduce(
                    out=nmA_part_all[:, b, h:h + 1],
                    in_=tAn, axis=AX.X, op=OP.max, negate=True)
            nc.vector.tensor_tensor(out=nmA_cols_all[:, b, :],
                                    in0=nmA_part_all[:, b, 0:1],
                                    in1=nmA_part_all[:, b, 1:2],
                                    op=OP.min)

        def nm_group(g0, gn):
            # anchor cols -> rows via PE transpose, then expand x8 with a
            # stride-0 free dim: -M_i = min(nm[i>>3], nm[(i>>3)+1]);
            # slot 128 = nm[127]-64 Lipschitz pad for the i>1016 tail.
            nmext_g = small.tile([4, 132], F32, tag="nmext")
            nm_row_g = small.tile([4, N], F32, tag="nmrow")
            nm_hilo_g = small.tile([4, 2, N], F32R, tag="nmhilo")
            tr = psum_an.tile([P, H], F32, tag="an")
            nc.tensor.matmul(tr[0:gn, 0:P], nmA_cols_all[:, g0:g0 + gn, 0],
                             idf, is_transpose=True, start=True, stop=True)
            nc.vector.tensor_copy(nmext_g[0:gn, 0:P], tr[0:gn, 0:P])
            nc.vector.tensor_scalar_add(nmext_g[0:gn, P:P + 1],
                                        nmext_g[0:gn, P - 1:P], -64.0)

            def rep8(off):
                base = nmext_g[0:gn, off:off + P]
                return bass.AP(tensor=base.tensor, offset=base.offset,
                               ap=list(base.ap) + [[0, 8]])

            nc.vector.tensor_tensor(
                out=nm_row_g[0:gn, :].rearrange("b (g r) -> b g r", r=8),
                in0=rep8(0), in1=rep8(1), op=OP.min)
            nc.vector.tensor_copy(nm_hilo_g[0:gn, 0, :], nm_row_g[0:gn, :])
            nc.vector.tensor_sub(nm_hilo_g[0:gn, 1, :], nm_row_g[0:gn, :],
                                 nm_hilo_g[0:gn, 0, :].bitcast(F32))
            for b in range(g0, g0 + gn):
                i = b - g0
                nc.sync.dma_start(out=rhs6_tiles[b % 4][4:5, :],
                                  in_=nm_hilo_g[i:i + 1, 0, :])
                nc.sync.dma_start(out=rhs6_tiles[b % 4][5:6, :],
                                  in_=nm_hilo_g[i:i + 1, 1, :])

        def phase3(b, companions=()):
            # tB[j,i] = a_i(s_hi+s_lo)_j - (r_hi+r_lo)_j - (m_hi+m_lo)_i
            # e = exp(tB); acc += [s_hi|s_lo|1]^T @ e.
            # companions: deferred ACT rowsum tiles of a later batch, woven
            # one-per-jt into the exp stream (ACT has ~1.5us slack per jt)
            # so they never delay the next batch's first exp.
            acc = psum_acc.tile([3, N], F32, tag="acc")
            comp = list(companions)
            for jt in range(NT):
                l6 = lhsT6_tiles[b % 4][:, jt * P:(jt + 1) * P]
                rhs6 = rhs6_tiles[b % 4]
                tB = psum_tb.tile([P, N], F32, tag="big")
                nc.tensor.matmul(tB[:, 0:H], l6, rhs6[:, 0:H],
                                 start=True, stop=True)
                nc.tensor.matmul(tB[:, H:N], l6, rhs6[:, H:N],
                                 start=True, stop=True)
                e = epool.tile([P, N], F32R, tag="e")
                nc.scalar.activation(out=e, in_=tB, func=ACT.Exp)
                if comp:
                    cb, cjt = comp.pop(0)
                    act_tile(cb, cjt)
                se_l = se_all[:, b, jt, :]
                nc.tensor.matmul(acc[:, 0:H], se_l, e[:, 0:H],
                                 start=(jt == 0), stop=(jt == NT - 1))
                nc.tensor.matmul(acc[:, H:N], se_l, e[:, H:N],
                                 start=(jt == 0), stop=(jt == NT - 1))
            nd_sb = small.tile([3, N], F32, tag="nd")
            nc.vector.tensor_copy(nd_sb, acc)
            nc.sync.dma_start(out=numer_hi_all[b:b + 1, :], in_=nd_sb[0:1, :])
            nc.sync.dma_start(out=numer_lo_all[b:b + 1, :], in_=nd_sb[1:2, :])
            nc.sync.dma_start(out=denom_all[b:b + 1, :], in_=nd_sb[2:3, :])

        # ---- schedule ----
        # broadcasts first on the SWDGE queue: b0 lands ~6us in, each next
        # ~4us later; every batch's rowsum starts well after its broadcast
        for b in range(BPC):
            bcast_dma(b, nc.gpsimd)
        splits_all()

        def chainA(b, act_tiles):
            rowsum(b, act_tiles=act_tiles)
            nr_split(b)
            make_rows(b)
            anchors(b)
            nm_group(b, 1)

        # interleave the two prologue chains: both rowsums issue first so
        # batch 1's nm chain lands before phase3(0) drains
        rowsum(0, act_tiles=(0, 1, 2, 3, 4))
        rowsum(1, act_tiles=(0, 1, 2, 3, 4, 5))
        nr_split(0)
        make_rows(0)
        anchors(0)
        nm_group(0, 1)
        nr_split(1)
        make_rows(1)
        anchors(1)
        nm_group(1, 1)

        def chainRest(b):
            nr_split(b)
            make_rows(b)
            anchors(b)
            nm_group(b, 1)

        # batch b: DVE rowsum tiles + deferred ACT tiles woven into
        # phase3(b-2)'s exp stream; the rest of the chain right after
        dq = rowsum(2, act_tiles=ACT_G2, defer=True)
        for b in range(2, BPC):
            phase3(b - 2, companions=dq)
            chainRest(b)
            if b + 1 < BPC:
                dq = rowsum(b + 1, act_tiles=ACT_G2, defer=True)
        phase3(BPC - 2, companions=dq)
        phase3(BPC - 1)

        # ---- epilogue ----
        numer_all = const.tile([BPC, N], F32)
        nc.vector.tensor_add(numer_all, numer_hi_all, numer_lo_all)
        rd = const.tile([BPC, N], F32)
        nc.vector.reciprocal(rd, denom_all)
        res = const.tile([BPC, N], F32)
        nc.vector.tensor_mul(res, numer_all, rd)
        nc.sync.dma_start(out=out_ext, in_=res)

    nc.compile()
    return nc


_CACHE = {}


def kernel(scores: np.ndarray) -> np.ndarray:
    scores = np.ascontiguousarray(scores, dtype=np.float32)
    assert scores.shape == (B, N)
    if "nc" not in _CACHE:
        _CACHE["nc"] = build_kernel()
    nc = _CACHE["nc"]

    a2d = (N + 1 - 2.0 * (np.arange(N, dtype=np.float32) + 1.0)).reshape(1, N)
    ones2d = np.ones((1, N), dtype=np.float32)
    in_maps = []
    for c in range(NCORES):
        in_maps.append({
            "scores": scores[c * BPC:(c + 1) * BPC],
            "avals": a2d,
            "onesv": ones2d,
        })
    r = run_bass_kernel_spmd(nc, in_maps, core_ids=list(range(NCORES)))
    out = np.concatenate([r.results[c]["out"] for c in range(NCORES)], axis=0)
    return out.astype(np.float32)


if __name__ == "__main__":
    x = np.random.randn(B, N).astype(np.float32)
    y = kernel(x)
    print(y.shape, y.dtype)
